# revision 33
# baseline (speedup 1.0000x reference)
"""Mistral-style MHA prefill kernel for Trainium2, 8-way tensor-parallel over heads.

Problem (hardcoded): B=1, S=2048, DIM=4096, 32 q-heads / 8 kv-heads, head_dim=128,
sliding window 2048 (== S, so the mask is exactly causal), rope theta 1e4.

Sharding: core c owns q-heads [4c, 4c+4) and kv-head c. wq/wk/wv are sharded on the
head axis, wo on its input (head) axis; each core computes a full-shape partial
output and the host sums the 8 partials (row-parallel linear + host all-reduce).

Layout strategy (all chosen host-side so the device never transposes activations):
  - x is passed pre-transposed xT [DIM, S]; projections run as W @ x -> [feat, S],
    so Q^T/K^T/V^T [128, S] per head come straight out of PSUM.
  - head_dim is permuted per 32-partition quadrant (16 re rows, then 16 im rows)
    so RoPE pairs sit +-16 apart inside a quadrant; 1/sqrt(head_dim) is folded
    into the rope tables as sqrt(scale). Q/K stay f32r so score accuracy holds.
  - scores are computed transposed, S_T[k, q] (k on partitions), per 512-wide
    q-block; exp runs on ScalarE into bf16 e tiles; the softmax denominator is
    a DVE bf16 running sum of e tiles (4x-rate all-SBUF adds) finished by ONE
    ones-matmul per block (vs one per k-tile), and 1/denom uses the fast
    approximate DVE reciprocal (~5x cheaper than the exact one).
  - P^T never needs a transpose: out^T[dv, q] accrues in PSUM with V (bf16,
    normal layout via 16 PE transposes) as the stationary operand.
  - causality at 128-col granularity; diagonal blocks masked with suffix slices
    of one [128, 512] zeros|triangle bf16 tile. All matmuls are N=512.
  - single fused pipeline: for each 512-row s-block, QKV projection -> RoPE ->
    attention q-block (all 4 heads; its keys are exactly s-blocks <= b) -> wo
    projection of those 512 output rows. Spreads exp/DVE/DMA load evenly and
    keeps the PE queue busy end to end.
  - dtypes: x/wqkv/wo/cs/otn/out-partials fp16 (halves DMA+SBUF, ~5e-4 rounding,
    matmul still 1 cyc/row); e/esum/V bf16 (e can reach ~6e4 > fp16 max);
    q/k/scores f32r. Host sums the 8 fp16 partial outputs in f32.
"""

import numpy as np

B = 1
S = 2048
DIM = 4096
N_HEADS = 32
N_KV = 8
DH = 128
NCORES = 8
HPC = N_HEADS // NCORES  # q heads per core
FQKV = HPC * DH + 2 * DH  # 768 projection rows per core
NKT = S // DH  # 16 k tiles
NQB = S // 512  # 4 q blocks
NDCH = DIM // DH  # 32 contraction chunks

_PROGRAM = None

# stream_shuffle mask: swap 16-partition halves within each 32-partition quadrant
_SWAP16 = [(i + 16) % 32 for i in range(32)]


def _head_perm():
    """Permutation of head_dim rows: quadrant q holds [re_16q..re_16q+15,
    im_16q..im_16q+15], so RoPE pairs are +-16 apart within a quadrant."""
    p = np.empty(DH, dtype=np.int64)
    for row in range(DH):
        q, j = divmod(row, 32)
        i = 16 * q + (j % 16)  # rope pair index
        p[row] = 2 * i + (0 if j < 16 else 1)
    return p


def _build_program():
    import concourse.bacc as bacc
    import concourse.mybir as mybir
    import concourse.tile as tile

    F32 = mybir.dt.float32
    F32R = mybir.dt.float32r
    F16 = mybir.dt.float16
    BF16 = mybir.dt.bfloat16
    EXP = mybir.ActivationFunctionType.Exp

    nc = bacc.Bacc("TRN2", target_bir_lowering=False, debug=False,
                   enable_asserts=False)

    xT = nc.dram_tensor("xT", [DIM, S], F16, kind="ExternalInput")
    # wqkv pre-arranged host-side into the exact SBUF image [128, 32*768] so
    # weight loads are a few large fully-contiguous DMAs
    wqkvT = nc.dram_tensor("wqkvT", [DH, NDCH * FQKV], F16, kind="ExternalInput")
    woT = nc.dram_tensor("woT", [HPC * DH, DIM], F16, kind="ExternalInput")
    csA_d = nc.dram_tensor("csA", [DH, S], F16, kind="ExternalInput")
    csB_d = nc.dram_tensor("csB", [DH, S], F16, kind="ExternalInput")
    sign_d = nc.dram_tensor("sign", [DH, 1], F32R, kind="ExternalInput")
    tri512_d = nc.dram_tensor("tri512", [DH, 512], BF16, kind="ExternalInput")
    ident_d = nc.dram_tensor("ident", [DH, DH], BF16, kind="ExternalInput")
    ones128_d = nc.dram_tensor("ones128", [DH, DH], BF16, kind="ExternalInput")
    out_d = nc.dram_tensor("out", [S, DIM], F16, kind="ExternalOutput")

    with tile.TileContext(nc) as tc, nc.allow_low_precision(
            reason="fp16/bf16 staging validated end-to-end vs f32 reference"):
        with (
            tc.tile_pool(name="consts", bufs=1) as cpool,
            tc.tile_pool(name="persist", bufs=1) as ppool,
        ):
            csA_sb = cpool.tile([DH, S], F16)
            csB_sb = cpool.tile([DH, S], F16)
            sign_sb = cpool.tile([DH, 1], F32R)
            tri512_sb = cpool.tile([DH, 512], BF16)
            ident_sb = cpool.tile([DH, DH], BF16)
            ones128_sb = cpool.tile([DH, DH], BF16)

            # q/k in fp16: same 1 cyc/row on the PE (and full rate even for
            # the 128-wide trimmed diagonal scores, unlike f32r), half the
            # SBUF, and it makes every rope op 2-byte all-SBUF so the DVE
            # runs the chains at 2-4x — the rope latency was what stalled
            # each block's first scores. fp16 rounding is ~5e-4 on q/k.
            qt = [ppool.tile([DH, S], F16, name=f"qt{h}") for h in range(HPC)]
            kt = ppool.tile([DH, S], F16)
            vn = ppool.tile([DH, S], BF16)  # V in normal layout, 16 [128,128] chunks
            otn = [ppool.tile([DH, S], F16, name=f"otn{h}") for h in range(HPC)]
            w_sb = ppool.tile([DH, NDCH * FQKV], F16)
            wo_sb = ppool.tile([DH, HPC * DIM], F16)

            with (
                tc.tile_pool(name="xin", bufs=8) as xpool,
                tc.tile_pool(name="mps", bufs=7, space="PSUM") as mps,
                tc.tile_pool(name="trps", bufs=1, space="PSUM") as trps,
                tc.tile_pool(name="ropet", bufs=2) as rtp,
                tc.tile_pool(name="rawsb", bufs=5) as rawpool,
                tc.tile_pool(name="vtt", bufs=1) as vtp,
                tc.tile_pool(name="esb", bufs=5) as epool,
                tc.tile_pool(name="essb", bufs=2) as espool,
                tc.tile_pool(name="bcsb", bufs=2) as bcpool,
                tc.tile_pool(name="evsb", bufs=3) as evpool,
            ):
                def emit_sblock(sb_i):
                    col = slice(sb_i * 512, (sb_i + 1) * 512)
                    ps = [mps.tile([DH, 512], F32, name=f"ps{f}", tag="ps")
                          for f in range(6)]
                    if sb_i == 0:
                        # first x half and first weight piece lead their HWDGE
                        # queues so the first matmul starts as early as
                        # possible; the remaining weights are a few big
                        # contiguous DMAs (the dram layout is the SBUF image)
                        # on the ACT ring, x alternates SP/SWDGE.
                        # first x halves lead the SP queue, weight pieces the
                        # ACT queue, so the first matmul's inputs arrive on
                        # two rings in parallel
                        x0 = xpool.tile([DH, 512], F16, name="xt", tag="xt")
                        nc.sync.dma_start(x0[:, 0:256], xT[0:DH, 0:256])
                        nc.scalar.dma_start(
                            w_sb[:, 0:DH], wqkvT[:, 0:DH])
                        nc.sync.dma_start(x0[:, 256:512], xT[0:DH, 256:512])
                        for f in range(1, 6):
                            nc.scalar.dma_start(
                                w_sb[:, f * DH:(f + 1) * DH],
                                wqkvT[:, f * DH:(f + 1) * DH])
                        for d0, d1 in [(1, 2), (2, 4), (4, 8), (8, 16),
                                       (16, 24), (24, 32)]:
                            nc.scalar.dma_start(
                                w_sb[:, d0 * FQKV:d1 * FQKV],
                                wqkvT[:, d0 * FQKV:d1 * FQKV])
                    for d in range(NDCH):
                        if sb_i == 0 and d == 0:
                            xt = x0
                        else:
                            xt = xpool.tile([DH, 512], F16, name="xt", tag="xt")
                            xeng = nc.sync if d % 2 == 0 else nc.gpsimd
                            xeng.dma_start(xt[:], xT[d * DH:(d + 1) * DH, col])
                        if sb_i == 0 and d == 12:
                            # constants are first needed by the sb0 ropes;
                            # don't let them delay the first matmuls
                            nc.gpsimd.dma_start(csA_sb[:], csA_d[:])
                            nc.gpsimd.dma_start(csB_sb[:], csB_d[:])
                            nc.gpsimd.dma_start(sign_sb[:], sign_d[:])
                        if sb_i == 0 and d == 20:
                            nc.gpsimd.dma_start(tri512_sb[:], tri512_d[:])
                            nc.gpsimd.dma_start(ident_sb[:], ident_d[:])
                            nc.gpsimd.dma_start(ones128_sb[:], ones128_d[:])
                        for f in range(6):
                            nc.tensor.matmul(
                                ps[f][:],
                                w_sb[:, d * FQKV + f * DH: d * FQKV + (f + 1) * DH],
                                xt[:], start=(d == 0), stop=(d == NDCH - 1))
                    if sb_i == 0:
                        # wo rides the ACT ring after the qkv weights; needed
                        # from the first wo half-groups (~55us in)
                        for ch in range(HPC):
                            nc.scalar.dma_start(
                                wo_sb[:, ch * DIM:(ch + 1) * DIM],
                                woT[ch * DH:(ch + 1) * DH, :])
                    # Fast raw PSUM->SBUF evictions (alternating ACT/DVE) free
                    # the accumulator banks quickly; RoPE runs later from SBUF.
                    # V first on ACT (cast to bf16, split per 128-chunk so the
                    # transposes can start off the first chunk); the PE
                    # transposes themselves are emitted later, as filler
                    # between the first block's scores (emit_transposes).
                    vt_t = vtp.tile([DH, 512], BF16, name="vt_t", tag="vt")
                    for t in range(4):
                        nc.scalar.copy(vt_t[:, t * DH:(t + 1) * DH],
                                       ps[5][:, t * DH:(t + 1) * DH])
                    # q0 first on DVE (its rope chain is the critical path
                    # into the first scores), K first on ACT; only ACT/DVE can
                    # read PSUM (GPSIMD cannot).
                    raws = {}
                    for i, f in enumerate([0, 4, 2, 1, 3]):
                        raw = rawpool.tile([DH, 512], F16, name="raw", tag="raw")
                        raws[f] = raw
                        if i % 2 == 1:
                            nc.scalar.copy(raw[:], ps[f][:])
                        else:
                            nc.vector.tensor_copy(raw[:], ps[f][:])

                    def emit_transposes():
                        # all four transposes land in one PSUM tile (disjoint
                        # column ranges, no inter-dependency), then a single
                        # wide DVE copy evicts to vn. Shares the 8th PSUM bank
                        # (tag aux) with the dn tiles — usage never overlaps.
                        tp = trps.tile([DH, 512], BF16, name="tp", tag="aux")
                        for t in range(4):
                            nc.tensor.transpose(
                                tp[:, t * DH:(t + 1) * DH],
                                vt_t[:, t * DH:(t + 1) * DH], ident_sb[:])
                        nc.vector.tensor_copy(
                            vn[:, sb_i * 512:(sb_i + 1) * 512], tp[:])
                    return raws, emit_transposes

                def emit_rope(f, sb_i, raw):
                    # head_dim permuted so pairs sit +-16 apart within each
                    # 32-partition quadrant: dest = p1 + sign*p3 where
                    # p1 = q*cos, p3 = halfswap(q)*sin.
                    col = slice(sb_i * 512, (sb_i + 1) * 512)
                    dest = qt[f] if f < HPC else kt
                    qs_t = rtp.tile([DH, 512], F16, name="qs_t", tag="qs")
                    p1 = rtp.tile([DH, 512], F16, name="p1", tag="p1")
                    nc.vector.stream_shuffle(qs_t[:], raw[:], _SWAP16)
                    nc.vector.tensor_mul(p1[:], raw[:], csA_sb[:, col])
                    nc.vector.tensor_mul(qs_t[:], qs_t[:], csB_sb[:, col])
                    nc.vector.scalar_tensor_tensor(
                        dest[:, col], qs_t[:], sign_sb[:], p1[:],
                        mybir.AluOpType.mult, mybir.AluOpType.add)

                def emit_block(h, b, pre=None, filler=None, sprinkle=None):
                    cb = slice(b * 512, (b + 1) * 512)
                    nk = 4 * b + 4  # k tiles contributing to this q block
                    ot_b = mps.tile([DH, 512], F32, name="ot", tag="ps")
                    esum = espool.tile([DH, 512], BF16, name="esum", tag="es")
                    e_tiles = [None] * nk
                    e_offs = [0] * nk

                    def emit_scores(k):
                        # diagonal k-tiles only see q >= 128*(k%4) of this
                        # block, so their score/exp/mask/sum ops are trimmed
                        # to the live suffix; after trimming the causal
                        # triangle always sits in the leading 128 columns.
                        off = 128 * (k % 4) if k // 4 == b else 0
                        n = 512 - off
                        e = epool.tile([DH, 512], BF16, name="E", tag="E")
                        e_tiles[k] = e
                        e_offs[k] = off
                        sp = mps.tile([DH, 512], F32, name="sp", tag="ps")
                        nc.tensor.matmul(
                            sp[:, :n], kt[:, k * DH:(k + 1) * DH],
                            qt[h][:, cb.start + off:cb.stop],
                            start=True, stop=True)
                        nc.scalar.activation(e[:, :n], sp[:, :n], EXP)
                        if k // 4 == b:
                            nc.vector.tensor_mul(
                                e[:, :DH], e[:, :DH], tri512_sb[:, 384:])
                        # bf16 all-SBUF running sum: 4x-rate DVE adds. The
                        # per-partition rounding noise averages out in the
                        # exact f32 PE partition-sum below.
                        if k == 1:
                            if off == 0:
                                nc.vector.tensor_add(esum[:], e_tiles[0][:],
                                                     e[:])
                            else:
                                # b=0: tile 1 is already trimmed; seed with
                                # tile 0 and add the live suffix
                                nc.vector.tensor_copy(esum[:], e_tiles[0][:])
                                nc.vector.tensor_add(esum[:, off:],
                                                     esum[:, off:], e[:, :n])
                        elif k > 1:
                            nc.vector.tensor_add(esum[:, off:],
                                                 esum[:, off:], e[:, :n])

                    def emit_pv(k):
                        e, off = e_tiles[k], e_offs[k]
                        st, sp_ = (k == 0), (k == nk - 1)
                        nc.tensor.matmul(ot_b[:, off:],
                                         vn[:, k * DH:(k + 1) * DH],
                                         e[:, :512 - off], start=st, stop=sp_,
                                         skip_group_check=(off > 0))

                    # 2-deep software pipeline: scores run two steps ahead of
                    # PV so exp/mask latency never stalls the PE. `pre` is
                    # ready PE work (deferred wo half-groups) placed before
                    # the first score, covering this block's rope latency;
                    # `filler` (prev block's dn chain, V transposes) slots
                    # behind the first two scores; `sprinkle(k)` injects the
                    # next head's rope and more ready work mid-block where the
                    # exp stream falls behind.
                    if pre is not None:
                        pre()
                    emit_scores(0)
                    emit_scores(1)
                    if filler is not None:
                        filler()
                    for k in range(2, nk):
                        emit_scores(k)
                        if sprinkle is not None:
                            sprinkle(k)
                        emit_pv(k - 2)
                    emit_pv(nk - 2)
                    emit_pv(nk - 1)

                    def finish():
                        # denominator: one ones-matmul over the summed e tiles
                        # broadcasts sum_k to every partition, then the fast
                        # approximate reciprocal (18 good bits) feeds the
                        # normalizing eviction into fp16 otn. Emitted later
                        # (next block / next s-block start) so the esum-tail
                        # latency hides behind ready PE work; the dn tile uses
                        # the 8th PSUM bank so it never contends with the six
                        # QKV accumulators.
                        dn_b = trps.tile([DH, 512], F32, name="dn", tag="aux")
                        nc.tensor.matmul(dn_b[:], ones128_sb[:], esum[:],
                                         start=True, stop=True)
                        bc_sb = bcpool.tile([DH, 512], F32, name="bc_sb",
                                            tag="bcs")
                        nc.vector.reciprocal_approx_fast(bc_sb[:], dn_b[:])
                        nc.vector.tensor_mul(otn[h][:, cb], ot_b[:], bc_sb[:])
                    return finish

                def make_wo_halfgroups(b, dn3=None):
                    """Half-groups (2 PSUM banks, 8 matmuls, one 1024-wide
                    fp16 stripe) of the wo projection for q-block b, returned
                    as closures to be doled out as PE filler across the next
                    s-block. dn3 (the h=3 dn chain) rides between the first
                    half-group's h=2 and h=3 accumulations."""
                    hgs = []
                    for st in range(4 * b, 4 * b + 4):
                        for q4 in range(4):  # 1024-wide output quarter
                            def hg(st=st, q4=q4,
                                   dn3=dn3 if not hgs else None):
                                scol = slice(st * DH, (st + 1) * DH)
                                base = q4 * 1024
                                pw = [mps.tile([DH, 512], F32, name=f"pw{j}",
                                               tag="ps") for j in range(2)]
                                for h in range(HPC):
                                    if h == HPC - 1 and dn3 is not None:
                                        dn3()
                                    for j in range(2):
                                        c0 = h * DIM + base + j * 512
                                        nc.tensor.matmul(
                                            pw[j][:], otn[h][:, scol],
                                            wo_sb[:, c0:c0 + 512],
                                            start=(h == 0), stop=(h == HPC - 1))
                                ev = evpool.tile([DH, 1024], F16, name="ev",
                                                 tag="ev")
                                if (st + q4) % 2 == 0:
                                    nc.scalar.copy(ev[:, 0:512], pw[0][:])
                                    nc.vector.tensor_copy(ev[:, 512:1024],
                                                          pw[1][:])
                                else:
                                    nc.vector.tensor_copy(ev[:, 0:512],
                                                          pw[0][:])
                                    nc.scalar.copy(ev[:, 512:1024], pw[1][:])
                                srow = slice(st * DH, (st + 1) * DH)
                                idx = (st - 4 * b) * 4 + q4
                                if b == NQB - 1 and idx == 15:
                                    # the very last stripe splits across both
                                    # HWDGE rings so the final transfer is
                                    # half as long (no SWDGE here: a late
                                    # SWDGE DMA costs ~7us of teardown drain)
                                    nc.sync.dma_start(
                                        out_d[srow, base:base + 512],
                                        ev[:, 0:512])
                                    nc.scalar.dma_start(
                                        out_d[srow, base + 512:base + 1024],
                                        ev[:, 512:1024])
                                elif (st + q4) % 2 == 0:
                                    nc.sync.dma_start(
                                        out_d[srow, base:base + 1024], ev[:])
                                else:
                                    nc.scalar.dma_start(
                                        out_d[srow, base:base + 1024], ev[:])
                            hgs.append(hg)
                    return hgs

                # ---- fused streaming pipeline over s-blocks ----
                # attention q-block b needs exactly K/V from s-blocks <= b, so
                # each s-block runs QKV -> rope -> attention (4 heads) back to
                # back; its wo projection is deferred into the NEXT s-block as
                # ready PE work covering every latency window there (rope
                # chains, esum tails, the exp stream inside long blocks).
                wo_q = []  # deferred wo half-group closures

                def take_wo(n):
                    if not wo_q:
                        return None

                    def run():
                        for hg in wo_q[:n]:
                            hg()
                        del wo_q[:n]
                    return run

                fin3 = None
                for sb_i in range(NQB):
                    if fin3 is not None:
                        # prev s-block's last dn/normalize: its esum is long
                        # done, so this runs stall-free and frees the prev ot
                        # PSUM bank ~1.5us into this QKV
                        fin3()
                        fin3 = None
                    raws, transposes = emit_sblock(sb_i)
                    take2 = take_wo(2)
                    if take2 is not None:
                        take2()  # right after QKV: covers raw-evict + q0 rope
                    emit_rope(0, sb_i, raws[0])
                    emit_rope(4, sb_i, raws[4])
                    fin_prev = None
                    for h in range(HPC):
                        def sprinkle(k, h=h, sb_i=sb_i):
                            if k == 2 and h + 1 < HPC:
                                # next head's rope rides early in this block's
                                # DVE stream so its first scores never wait
                                emit_rope(h + 1, sb_i, raws[h + 1])
                            if k % 5 == 4:
                                t = take_wo(1)
                                if t is not None:
                                    t()  # keep the PE fed where exp lags
                        if h == 0:
                            # sb0 has no deferred wo to fill the rope-latency
                            # window, so run the transposes first instead of
                            # idling before the first score
                            pre = transposes if sb_i == 0 else None
                            filler = None if sb_i == 0 else transposes
                        else:
                            pre = take_wo(1)
                            filler = fin_prev
                        fin_prev = emit_block(h, sb_i, pre=pre, filler=filler,
                                              sprinkle=sprinkle)
                    # flush any leftover deferred half-groups before the next
                    # s-block's QKV claims all six accumulator banks
                    flush = take_wo(len(wo_q))
                    if flush is not None:
                        flush()
                    fin3 = fin_prev
                    if sb_i < NQB - 1:
                        wo_q.extend(make_wo_halfgroups(sb_i))
                    else:
                        wo_q.extend(make_wo_halfgroups(sb_i, dn3=fin3))
                        fin3 = None
                # tail: the last s-block's wo half-groups run back to back
                for hg in wo_q:
                    hg()

    nc.compile()
    return nc


def get_program():
    global _PROGRAM
    if _PROGRAM is None:
        _PROGRAM = _build_program()
    return _PROGRAM


def make_in_maps(inputs):
    """Host-side sharding / layout prep. Returns one input dict per core."""
    import ml_dtypes
    bf16 = ml_dtypes.bfloat16

    x = np.asarray(inputs["x"], dtype=np.float32)
    wq = np.asarray(inputs["wq"], dtype=np.float32)
    wk = np.asarray(inputs["wk"], dtype=np.float32)
    wv = np.asarray(inputs["wv"], dtype=np.float32)
    wo = np.asarray(inputs["wo"], dtype=np.float32)
    cos = np.asarray(inputs["freqs_cos"], dtype=np.float32)  # (S, 64)
    sin = np.asarray(inputs["freqs_sin"], dtype=np.float32)

    xT = np.ascontiguousarray(x.reshape(S, DIM).T).astype(np.float16)  # (DIM, S)

    perm = _head_perm()
    sq = np.float32(DH ** -0.25)  # sqrt of 1/sqrt(head_dim), folded into Q and K
    rows = np.arange(DH)
    pair_idx = 16 * (rows // 32) + (rows % 32) % 16
    csA = np.ascontiguousarray(cos.T[pair_idx] * sq).astype(np.float16)  # (128, S)
    csB = np.ascontiguousarray(sin.T[pair_idx] * sq).astype(np.float16)
    sign = np.where((rows % 32) < 16, -1.0, 1.0).astype(np.float32).reshape(DH, 1)
    tri = np.triu(np.ones((DH, DH), dtype=np.float32))
    tri512 = np.concatenate([np.zeros((DH, 512 - DH), np.float32), tri],
                            axis=1).astype(bf16)
    ident = np.eye(DH, dtype=np.float32).astype(bf16)
    ones128 = np.ones((DH, DH), dtype=np.float32).astype(bf16)

    wqh = wq.reshape(N_HEADS, DH, DIM)[:, perm, :]
    wkh = wk.reshape(N_KV, DH, DIM)[:, perm, :]
    wvh = wv.reshape(N_KV, DH, DIM)

    in_maps = []
    for c in range(NCORES):
        w_c = np.concatenate(
            [wqh[HPC * c:HPC * (c + 1)].reshape(HPC * DH, DIM),
             wkh[c], wvh[c]], 0)  # (768, DIM)
        # SBUF image [128, 32*768]: partition p holds chunk-d cols at
        # [d*768, (d+1)*768) = w_c[:, d*128+p] for each d
        wqkvT = np.ascontiguousarray(
            w_c.T.reshape(NDCH, DH, FQKV).transpose(1, 0, 2).reshape(
                DH, NDCH * FQKV)).astype(np.float16)
        woT = np.ascontiguousarray(
            wo[:, HPC * DH * c:HPC * DH * (c + 1)].T).astype(np.float16)
        in_maps.append({
            "xT": xT, "wqkvT": wqkvT, "woT": woT,
            "csA": csA, "csB": csB, "sign": sign, "tri512": tri512,
            "ident": ident, "ones128": ones128,
        })
    return in_maps


def _ensure_ntff_hook():
    """The agent image's `antenv` lacks `axon_hooks`; recreate it so
    run_bass_kernel_spmd(trace=True) can capture NTFF profiles. Mirrors
    trn_agent_boot/trn_boot.py::_ntff_profile_via_ctypes."""
    import sys
    try:
        from antenv.axon_hooks import get_axon_ntff_profile_hook  # noqa: F401
        return
    except ImportError:
        pass
    import contextlib
    import ctypes
    import types

    so_path = "/opt/axon/libaxon_pjrt.so"
    hook = None
    try:
        lib = ctypes.CDLL(so_path)
        if hasattr(lib, "axon_start_nrt_profile"):
            lib.axon_start_nrt_profile.argtypes = [
                ctypes.POINTER(ctypes.c_int64), ctypes.c_size_t]
            lib.axon_start_nrt_profile.restype = ctypes.c_int64
            lib.axon_stop_nrt_profile.argtypes = [ctypes.c_char_p]
            lib.axon_stop_nrt_profile.restype = ctypes.c_int64

            @contextlib.contextmanager
            def _hook(output_dir, device_ids):
                import jax
                jax.devices()
                if device_ids:
                    ids = (ctypes.c_int64 * len(device_ids))(*device_ids)
                    rc = lib.axon_start_nrt_profile(ids, len(device_ids))
                else:
                    rc = lib.axon_start_nrt_profile(None, 0)
                if rc != 0:
                    raise RuntimeError(f"axon_start_nrt_profile rc={rc}")
                try:
                    yield
                finally:
                    n = lib.axon_stop_nrt_profile(str(output_dir).encode())
                    print(f"profile: {n} file(s) written to {output_dir}")

            hook = _hook
    except OSError:
        pass

    mod = types.ModuleType("antenv.axon_hooks")
    mod._hook = hook
    mod.get_axon_ntff_profile_hook = lambda: mod._hook
    mod.set_axon_ntff_profile_hook = lambda h: setattr(mod, "_hook", h)
    sys.modules["antenv.axon_hooks"] = mod


def run(inputs, trace=False):
    from concourse.bass_utils import run_bass_kernel_spmd
    if trace:
        _ensure_ntff_hook()
    nc = get_program()
    in_maps = make_in_maps(inputs)
    res = run_bass_kernel_spmd(nc, in_maps, core_ids=list(range(NCORES)),
                               trace=trace)
    acc = np.zeros((S, DIM), dtype=np.float32)
    for r in res.results:
        acc += np.asarray(r["out"], dtype=np.float32)
    return acc.reshape(B, S, DIM), res


def kernel(**inputs):
    out, _ = run(inputs, trace=False)
    return out


# revision 36
# speedup vs baseline: 1.0353x; 1.0353x over previous
"""Mistral-style MHA prefill kernel for Trainium2, 8-way tensor-parallel over heads.

Problem (hardcoded): B=1, S=2048, DIM=4096, 32 q-heads / 8 kv-heads, head_dim=128,
sliding window 2048 (== S, so the mask is exactly causal), rope theta 1e4.

Sharding: core c owns q-heads [4c, 4c+4) and kv-head c. wq/wk/wv are sharded on the
head axis, wo on its input (head) axis; each core computes a full-shape partial
output and the host sums the 8 partials (row-parallel linear + host all-reduce).

Layout strategy (all chosen host-side so the device never transposes activations):
  - x is passed pre-transposed xT [DIM, S]; projections run as W @ x -> [feat, S],
    so Q^T/K^T/V^T [128, S] per head come straight out of PSUM.
  - head_dim is permuted per 32-partition quadrant (16 re rows, then 16 im rows)
    so RoPE pairs sit +-16 apart inside a quadrant; 1/sqrt(head_dim) is folded
    into the rope tables as sqrt(scale). Q/K stay f32r so score accuracy holds.
  - scores are computed transposed, S_T[k, q] (k on partitions), per 512-wide
    q-block; exp runs on ScalarE into bf16 e tiles; the softmax denominator is
    a DVE bf16 running sum of e tiles (4x-rate all-SBUF adds) finished by ONE
    ones-matmul per block (vs one per k-tile), and 1/denom uses the fast
    approximate DVE reciprocal (~5x cheaper than the exact one).
  - P^T never needs a transpose: out^T[dv, q] accrues in PSUM with V (bf16,
    normal layout via 16 PE transposes) as the stationary operand.
  - causality at 128-col granularity; diagonal blocks masked with suffix slices
    of one [128, 512] zeros|triangle bf16 tile. All matmuls are N=512.
  - single fused pipeline: for each 512-row s-block, QKV projection -> RoPE ->
    attention q-block (all 4 heads; its keys are exactly s-blocks <= b) -> wo
    projection of those 512 output rows. Spreads exp/DVE/DMA load evenly and
    keeps the PE queue busy end to end.
  - dtypes: x/wqkv/wo/cs/otn/out-partials fp16 (halves DMA+SBUF, ~5e-4 rounding,
    matmul still 1 cyc/row); e/esum/V bf16 (e can reach ~6e4 > fp16 max);
    q/k/scores f32r. Host sums the 8 fp16 partial outputs in f32.
"""

import numpy as np

B = 1
S = 2048
DIM = 4096
N_HEADS = 32
N_KV = 8
DH = 128
NCORES = 8
HPC = N_HEADS // NCORES  # q heads per core
FQKV = HPC * DH + 2 * DH  # 768 projection rows per core
NKT = S // DH  # 16 k tiles
NQB = S // 512  # 4 q blocks
NDCH = DIM // DH  # 32 contraction chunks

_PROGRAM = None

# stream_shuffle mask: swap 16-partition halves within each 32-partition quadrant
_SWAP16 = [(i + 16) % 32 for i in range(32)]


def _head_perm():
    """Permutation of head_dim rows: quadrant q holds [re_16q..re_16q+15,
    im_16q..im_16q+15], so RoPE pairs are +-16 apart within a quadrant."""
    p = np.empty(DH, dtype=np.int64)
    for row in range(DH):
        q, j = divmod(row, 32)
        i = 16 * q + (j % 16)  # rope pair index
        p[row] = 2 * i + (0 if j < 16 else 1)
    return p


def _build_program():
    import concourse.bacc as bacc
    import concourse.mybir as mybir
    import concourse.tile as tile

    F32 = mybir.dt.float32
    F32R = mybir.dt.float32r
    F16 = mybir.dt.float16
    BF16 = mybir.dt.bfloat16
    EXP = mybir.ActivationFunctionType.Exp

    nc = bacc.Bacc("TRN2", target_bir_lowering=False, debug=False,
                   enable_asserts=False)

    xT = nc.dram_tensor("xT", [DIM, S], F16, kind="ExternalInput")
    # wqkv pre-arranged host-side into the exact SBUF image [128, 32*768] so
    # weight loads are a few large fully-contiguous DMAs
    wqkvT = nc.dram_tensor("wqkvT", [DH, NDCH * FQKV], F16, kind="ExternalInput")
    woT = nc.dram_tensor("woT", [HPC * DH, DIM], F16, kind="ExternalInput")
    csA_d = nc.dram_tensor("csA", [DH, S], F16, kind="ExternalInput")
    csB_d = nc.dram_tensor("csB", [DH, S], F16, kind="ExternalInput")
    sign_d = nc.dram_tensor("sign", [DH, 1], F32R, kind="ExternalInput")
    tri512_d = nc.dram_tensor("tri512", [DH, 512], BF16, kind="ExternalInput")
    ident_d = nc.dram_tensor("ident", [DH, DH], BF16, kind="ExternalInput")
    ones128_d = nc.dram_tensor("ones128", [DH, DH], BF16, kind="ExternalInput")
    out_d = nc.dram_tensor("out", [S, DIM], F16, kind="ExternalOutput")

    with tile.TileContext(nc) as tc, nc.allow_low_precision(
            reason="fp16/bf16 staging validated end-to-end vs f32 reference"):
        with (
            tc.tile_pool(name="consts", bufs=1) as cpool,
            tc.tile_pool(name="persist", bufs=1) as ppool,
        ):
            csA_sb = cpool.tile([DH, S], F16)
            csB_sb = cpool.tile([DH, S], F16)
            sign_sb = cpool.tile([DH, 1], F32R)
            tri512_sb = cpool.tile([DH, 512], BF16)
            ident_sb = cpool.tile([DH, DH], BF16)
            ones128_sb = cpool.tile([DH, DH], BF16)

            qt = [ppool.tile([DH, S], F32R, name=f"qt{h}") for h in range(HPC)]
            kt = ppool.tile([DH, S], F32R)
            vn = ppool.tile([DH, S], BF16)  # V in normal layout, 16 [128,128] chunks
            otn = [ppool.tile([DH, S], F16, name=f"otn{h}") for h in range(HPC)]
            w_sb = ppool.tile([DH, NDCH * FQKV], F16)
            wo_sb = ppool.tile([DH, HPC * DIM], F16)

            with (
                tc.tile_pool(name="xin", bufs=8) as xpool,
                tc.tile_pool(name="mps", bufs=7, space="PSUM") as mps,
                tc.tile_pool(name="trps", bufs=1, space="PSUM") as trps,
                tc.tile_pool(name="ropet", bufs=2) as rtp,
                tc.tile_pool(name="rawsb", bufs=5) as rawpool,
                tc.tile_pool(name="vtt", bufs=1) as vtp,
                tc.tile_pool(name="esb", bufs=5) as epool,
                tc.tile_pool(name="essb", bufs=2) as espool,
                tc.tile_pool(name="bcsb", bufs=2) as bcpool,
                tc.tile_pool(name="evsb", bufs=3) as evpool,
            ):
                def emit_sblock(sb_i):
                    col = slice(sb_i * 512, (sb_i + 1) * 512)
                    ps = [mps.tile([DH, 512], F32, name=f"ps{f}", tag="ps")
                          for f in range(6)]
                    if sb_i == 0:
                        # first x half and first weight piece lead their HWDGE
                        # queues so the first matmul starts as early as
                        # possible; the remaining weights are a few big
                        # contiguous DMAs (the dram layout is the SBUF image)
                        # on the ACT ring, x alternates SP/SWDGE.
                        # the critical first tiles (x0 halves + w piece 0) all
                        # lead the SP queue — the ACT queue starts ~2us late
                        # (activation-table preamble)
                        x0 = xpool.tile([DH, 512], F16, name="xt", tag="xt")
                        nc.sync.dma_start(x0[:, 0:256], xT[0:DH, 0:256])
                        nc.sync.dma_start(
                            w_sb[:, 0:DH], wqkvT[:, 0:DH])
                        nc.sync.dma_start(x0[:, 256:512], xT[0:DH, 256:512])
                        nc.sync.dma_start(w_sb[:, DH:2 * DH],
                                          wqkvT[:, DH:2 * DH])
                        for f in range(2, 6):
                            nc.scalar.dma_start(
                                w_sb[:, f * DH:(f + 1) * DH],
                                wqkvT[:, f * DH:(f + 1) * DH])
                        for d0, d1 in [(1, 2), (2, 4), (4, 8), (8, 16),
                                       (16, 24), (24, 32)]:
                            nc.scalar.dma_start(
                                w_sb[:, d0 * FQKV:d1 * FQKV],
                                wqkvT[:, d0 * FQKV:d1 * FQKV])
                    for d in range(NDCH):
                        if sb_i == 0 and d == 0:
                            xt = x0
                        else:
                            xt = xpool.tile([DH, 512], F16, name="xt", tag="xt")
                            xeng = nc.sync if d % 2 == 0 else nc.gpsimd
                            xeng.dma_start(xt[:], xT[d * DH:(d + 1) * DH, col])
                        if sb_i == 0 and d == 12:
                            # constants are first needed by the sb0 ropes;
                            # don't let them delay the first matmuls
                            nc.gpsimd.dma_start(csA_sb[:], csA_d[:])
                            nc.gpsimd.dma_start(csB_sb[:], csB_d[:])
                            nc.gpsimd.dma_start(sign_sb[:], sign_d[:])
                        if sb_i == 0 and d == 20:
                            nc.gpsimd.dma_start(tri512_sb[:], tri512_d[:])
                            nc.gpsimd.dma_start(ident_sb[:], ident_d[:])
                            nc.gpsimd.dma_start(ones128_sb[:], ones128_d[:])
                        for f in range(6):
                            nc.tensor.matmul(
                                ps[f][:],
                                w_sb[:, d * FQKV + f * DH: d * FQKV + (f + 1) * DH],
                                xt[:], start=(d == 0), stop=(d == NDCH - 1))
                    if sb_i == 0:
                        # wo rides the ACT ring after the qkv weights; needed
                        # from the first wo half-groups (~55us in)
                        for ch in range(HPC):
                            nc.scalar.dma_start(
                                wo_sb[:, ch * DIM:(ch + 1) * DIM],
                                woT[ch * DH:(ch + 1) * DH, :])
                    # Fast raw PSUM->SBUF evictions (alternating ACT/DVE) free
                    # the accumulator banks quickly; RoPE runs later from SBUF.
                    # V first on ACT (cast to bf16, split per 128-chunk so the
                    # transposes can start off the first chunk); the PE
                    # transposes themselves are emitted later, as filler
                    # between the first block's scores (emit_transposes).
                    vt_t = vtp.tile([DH, 512], BF16, name="vt_t", tag="vt")
                    for t in range(4):
                        nc.scalar.copy(vt_t[:, t * DH:(t + 1) * DH],
                                       ps[5][:, t * DH:(t + 1) * DH])
                    # q0 first on DVE (its rope chain is the critical path
                    # into the first scores), K first on ACT; only ACT/DVE can
                    # read PSUM (GPSIMD cannot).
                    # raws evicted as fp16: the rope chain then runs on 2-byte
                    # all-SBUF operands (2-4x DVE rate) — it is the latency
                    # chain gating each block's first scores. q/k/scores stay
                    # f32r: fp16 MATMULS measured slower on real HW.
                    raws = {}
                    for i, f in enumerate([0, 4, 2, 1, 3]):
                        raw = rawpool.tile([DH, 512], F16, name="raw", tag="raw")
                        raws[f] = raw
                        if i % 2 == 1:
                            nc.scalar.copy(raw[:], ps[f][:])
                        else:
                            nc.vector.tensor_copy(raw[:], ps[f][:])

                    def emit_transposes():
                        # all four transposes land in one PSUM tile (disjoint
                        # column ranges, no inter-dependency), then a single
                        # wide DVE copy evicts to vn. Shares the 8th PSUM bank
                        # (tag aux) with the dn tiles — usage never overlaps.
                        tp = trps.tile([DH, 512], BF16, name="tp", tag="aux")
                        for t in range(4):
                            nc.tensor.transpose(
                                tp[:, t * DH:(t + 1) * DH],
                                vt_t[:, t * DH:(t + 1) * DH], ident_sb[:])
                        nc.vector.tensor_copy(
                            vn[:, sb_i * 512:(sb_i + 1) * 512], tp[:])
                    return raws, emit_transposes

                def emit_rope(f, sb_i, raw):
                    # head_dim permuted so pairs sit +-16 apart within each
                    # 32-partition quadrant: dest = p1 + sign*p3 where
                    # p1 = q*cos, p3 = halfswap(q)*sin.
                    col = slice(sb_i * 512, (sb_i + 1) * 512)
                    dest = qt[f] if f < HPC else kt
                    qs_t = rtp.tile([DH, 512], F16, name="qs_t", tag="qs")
                    p1 = rtp.tile([DH, 512], F16, name="p1", tag="p1")
                    nc.vector.stream_shuffle(qs_t[:], raw[:], _SWAP16)
                    nc.vector.tensor_mul(p1[:], raw[:], csA_sb[:, col])
                    nc.vector.tensor_mul(qs_t[:], qs_t[:], csB_sb[:, col])
                    nc.vector.scalar_tensor_tensor(
                        dest[:, col], qs_t[:], sign_sb[:], p1[:],
                        mybir.AluOpType.mult, mybir.AluOpType.add)

                def emit_block(h, b, pre=None, filler=None, sprinkle=None):
                    cb = slice(b * 512, (b + 1) * 512)
                    nk = 4 * b + 4  # k tiles contributing to this q block
                    ot_b = mps.tile([DH, 512], F32, name="ot", tag="ps")
                    esum = espool.tile([DH, 512], BF16, name="esum", tag="es")
                    e_tiles = [None] * nk
                    e_offs = [0] * nk

                    def emit_scores(k):
                        # diagonal k-tiles only see q >= 128*(k%4) of this
                        # block, so their score/exp/mask/sum ops are trimmed
                        # to the live suffix; after trimming the causal
                        # triangle always sits in the leading 128 columns.
                        off = 128 * (k % 4) if k // 4 == b else 0
                        n = 512 - off
                        e = epool.tile([DH, 512], BF16, name="E", tag="E")
                        e_tiles[k] = e
                        e_offs[k] = off
                        sp = mps.tile([DH, 512], F32, name="sp", tag="ps")
                        nc.tensor.matmul(
                            sp[:, :n], kt[:, k * DH:(k + 1) * DH],
                            qt[h][:, cb.start + off:cb.stop],
                            start=True, stop=True)
                        nc.scalar.activation(e[:, :n], sp[:, :n], EXP)
                        if k // 4 == b:
                            nc.vector.tensor_mul(
                                e[:, :DH], e[:, :DH], tri512_sb[:, 384:])
                        # bf16 all-SBUF running sum: 4x-rate DVE adds. The
                        # per-partition rounding noise averages out in the
                        # exact f32 PE partition-sum below.
                        if k == 1:
                            if off == 0:
                                nc.vector.tensor_add(esum[:], e_tiles[0][:],
                                                     e[:])
                            else:
                                # b=0: tile 1 is already trimmed; seed with
                                # tile 0 and add the live suffix
                                nc.vector.tensor_copy(esum[:], e_tiles[0][:])
                                nc.vector.tensor_add(esum[:, off:],
                                                     esum[:, off:], e[:, :n])
                        elif k > 1:
                            nc.vector.tensor_add(esum[:, off:],
                                                 esum[:, off:], e[:, :n])

                    def emit_pv(k):
                        e, off = e_tiles[k], e_offs[k]
                        st, sp_ = (k == 0), (k == nk - 1)
                        nc.tensor.matmul(ot_b[:, off:],
                                         vn[:, k * DH:(k + 1) * DH],
                                         e[:, :512 - off], start=st, stop=sp_,
                                         skip_group_check=(off > 0))

                    # 2-deep software pipeline: scores run two steps ahead of
                    # PV so exp/mask latency never stalls the PE. `pre` is
                    # ready PE work (deferred wo half-groups) placed before
                    # the first score, covering this block's rope latency;
                    # `filler` (prev block's dn chain, V transposes) slots
                    # behind the first two scores; `sprinkle(k)` injects the
                    # next head's rope and more ready work mid-block where the
                    # exp stream falls behind.
                    if pre is not None:
                        pre()
                    emit_scores(0)
                    emit_scores(1)
                    if filler is not None:
                        filler()
                    for k in range(2, nk):
                        emit_scores(k)
                        if sprinkle is not None:
                            sprinkle(k)
                        emit_pv(k - 2)
                    emit_pv(nk - 2)
                    emit_pv(nk - 1)

                    def finish():
                        # denominator: one ones-matmul over the summed e tiles
                        # broadcasts sum_k to every partition, then the fast
                        # approximate reciprocal (18 good bits) feeds the
                        # normalizing eviction into fp16 otn. Emitted later
                        # (next block / next s-block start) so the esum-tail
                        # latency hides behind ready PE work; the dn tile uses
                        # the 8th PSUM bank so it never contends with the six
                        # QKV accumulators.
                        dn_b = trps.tile([DH, 512], F32, name="dn", tag="aux")
                        nc.tensor.matmul(dn_b[:], ones128_sb[:], esum[:],
                                         start=True, stop=True)
                        bc_sb = bcpool.tile([DH, 512], F32, name="bc_sb",
                                            tag="bcs")
                        nc.vector.reciprocal_approx_fast(bc_sb[:], dn_b[:])
                        nc.vector.tensor_mul(otn[h][:, cb], ot_b[:], bc_sb[:])
                    return finish

                def make_wo_halfgroups(b, dn3=None):
                    """Half-groups (2 PSUM banks, 8 matmuls, one 1024-wide
                    fp16 stripe) of the wo projection for q-block b, returned
                    as closures to be doled out as PE filler across the next
                    s-block. dn3 (the h=3 dn chain) rides between the first
                    half-group's h=2 and h=3 accumulations."""
                    hgs = []
                    for st in range(4 * b, 4 * b + 4):
                        for q4 in range(4):  # 1024-wide output quarter
                            def hg(st=st, q4=q4,
                                   dn3=dn3 if not hgs else None):
                                scol = slice(st * DH, (st + 1) * DH)
                                base = q4 * 1024
                                pw = [mps.tile([DH, 512], F32, name=f"pw{j}",
                                               tag="ps") for j in range(2)]
                                for h in range(HPC):
                                    if h == HPC - 1 and dn3 is not None:
                                        dn3()
                                    for j in range(2):
                                        c0 = h * DIM + base + j * 512
                                        nc.tensor.matmul(
                                            pw[j][:], otn[h][:, scol],
                                            wo_sb[:, c0:c0 + 512],
                                            start=(h == 0), stop=(h == HPC - 1))
                                ev = evpool.tile([DH, 1024], F16, name="ev",
                                                 tag="ev")
                                if (st + q4) % 2 == 0:
                                    nc.scalar.copy(ev[:, 0:512], pw[0][:])
                                    nc.vector.tensor_copy(ev[:, 512:1024],
                                                          pw[1][:])
                                else:
                                    nc.vector.tensor_copy(ev[:, 0:512],
                                                          pw[0][:])
                                    nc.scalar.copy(ev[:, 512:1024], pw[1][:])
                                srow = slice(st * DH, (st + 1) * DH)
                                idx = (st - 4 * b) * 4 + q4
                                if b == NQB - 1 and idx == 15:
                                    # the very last stripe splits across both
                                    # HWDGE rings so the final transfer is
                                    # half as long (no SWDGE here: a late
                                    # SWDGE DMA costs ~7us of teardown drain)
                                    nc.sync.dma_start(
                                        out_d[srow, base:base + 512],
                                        ev[:, 0:512])
                                    nc.scalar.dma_start(
                                        out_d[srow, base + 512:base + 1024],
                                        ev[:, 512:1024])
                                elif (st + q4) % 2 == 0:
                                    nc.sync.dma_start(
                                        out_d[srow, base:base + 1024], ev[:])
                                else:
                                    nc.scalar.dma_start(
                                        out_d[srow, base:base + 1024], ev[:])
                            hgs.append(hg)
                    return hgs

                # ---- fused streaming pipeline over s-blocks ----
                # attention q-block b needs exactly K/V from s-blocks <= b, so
                # each s-block runs QKV -> rope -> attention (4 heads) back to
                # back; its wo projection is deferred into the NEXT s-block as
                # ready PE work covering every latency window there (rope
                # chains, esum tails, the exp stream inside long blocks).
                wo_q = []  # deferred wo half-group closures

                def take_wo(n):
                    if not wo_q:
                        return None

                    def run():
                        for hg in wo_q[:n]:
                            hg()
                        del wo_q[:n]
                    return run

                fin3 = None
                for sb_i in range(NQB):
                    if fin3 is not None:
                        # prev s-block's last dn/normalize: its esum is long
                        # done, so this runs stall-free and frees the prev ot
                        # PSUM bank ~1.5us into this QKV
                        fin3()
                        fin3 = None
                    raws, transposes = emit_sblock(sb_i)
                    take2 = take_wo(2)
                    if take2 is not None:
                        take2()  # right after QKV: covers raw-evict + q0 rope
                    emit_rope(0, sb_i, raws[0])
                    emit_rope(4, sb_i, raws[4])
                    fin_prev = None
                    for h in range(HPC):
                        def sprinkle(k, h=h, sb_i=sb_i):
                            if k == 2 and h + 1 < HPC:
                                # next head's rope rides early in this block's
                                # DVE stream so its first scores never wait
                                emit_rope(h + 1, sb_i, raws[h + 1])
                            if k % 5 == 4:
                                t = take_wo(1)
                                if t is not None:
                                    t()  # keep the PE fed where exp lags
                        if h == 0:
                            # sb0 has no deferred wo to fill the rope-latency
                            # window, so run the transposes first instead of
                            # idling before the first score
                            pre = transposes if sb_i == 0 else None
                            filler = None if sb_i == 0 else transposes
                        else:
                            pre = take_wo(1)
                            filler = fin_prev
                        fin_prev = emit_block(h, sb_i, pre=pre, filler=filler,
                                              sprinkle=sprinkle)
                    # flush any leftover deferred half-groups before the next
                    # s-block's QKV claims all six accumulator banks
                    flush = take_wo(len(wo_q))
                    if flush is not None:
                        flush()
                    fin3 = fin_prev
                    if sb_i < NQB - 1:
                        wo_q.extend(make_wo_halfgroups(sb_i))
                    else:
                        wo_q.extend(make_wo_halfgroups(sb_i, dn3=fin3))
                        fin3 = None
                # tail: the last s-block's wo half-groups run back to back
                for hg in wo_q:
                    hg()

    nc.compile()
    return nc


def get_program():
    global _PROGRAM
    if _PROGRAM is None:
        _PROGRAM = _build_program()
    return _PROGRAM


def make_in_maps(inputs):
    """Host-side sharding / layout prep. Returns one input dict per core."""
    import ml_dtypes
    bf16 = ml_dtypes.bfloat16

    x = np.asarray(inputs["x"], dtype=np.float32)
    wq = np.asarray(inputs["wq"], dtype=np.float32)
    wk = np.asarray(inputs["wk"], dtype=np.float32)
    wv = np.asarray(inputs["wv"], dtype=np.float32)
    wo = np.asarray(inputs["wo"], dtype=np.float32)
    cos = np.asarray(inputs["freqs_cos"], dtype=np.float32)  # (S, 64)
    sin = np.asarray(inputs["freqs_sin"], dtype=np.float32)

    xT = np.ascontiguousarray(x.reshape(S, DIM).T).astype(np.float16)  # (DIM, S)

    perm = _head_perm()
    sq = np.float32(DH ** -0.25)  # sqrt of 1/sqrt(head_dim), folded into Q and K
    rows = np.arange(DH)
    pair_idx = 16 * (rows // 32) + (rows % 32) % 16
    csA = np.ascontiguousarray(cos.T[pair_idx] * sq).astype(np.float16)  # (128, S)
    csB = np.ascontiguousarray(sin.T[pair_idx] * sq).astype(np.float16)
    sign = np.where((rows % 32) < 16, -1.0, 1.0).astype(np.float32).reshape(DH, 1)
    tri = np.triu(np.ones((DH, DH), dtype=np.float32))
    tri512 = np.concatenate([np.zeros((DH, 512 - DH), np.float32), tri],
                            axis=1).astype(bf16)
    ident = np.eye(DH, dtype=np.float32).astype(bf16)
    ones128 = np.ones((DH, DH), dtype=np.float32).astype(bf16)

    wqh = wq.reshape(N_HEADS, DH, DIM)[:, perm, :]
    wkh = wk.reshape(N_KV, DH, DIM)[:, perm, :]
    wvh = wv.reshape(N_KV, DH, DIM)

    in_maps = []
    for c in range(NCORES):
        w_c = np.concatenate(
            [wqh[HPC * c:HPC * (c + 1)].reshape(HPC * DH, DIM),
             wkh[c], wvh[c]], 0)  # (768, DIM)
        # SBUF image [128, 32*768]: partition p holds chunk-d cols at
        # [d*768, (d+1)*768) = w_c[:, d*128+p] for each d
        wqkvT = np.ascontiguousarray(
            w_c.T.reshape(NDCH, DH, FQKV).transpose(1, 0, 2).reshape(
                DH, NDCH * FQKV)).astype(np.float16)
        woT = np.ascontiguousarray(
            wo[:, HPC * DH * c:HPC * DH * (c + 1)].T).astype(np.float16)
        in_maps.append({
            "xT": xT, "wqkvT": wqkvT, "woT": woT,
            "csA": csA, "csB": csB, "sign": sign, "tri512": tri512,
            "ident": ident, "ones128": ones128,
        })
    return in_maps


def _ensure_ntff_hook():
    """The agent image's `antenv` lacks `axon_hooks`; recreate it so
    run_bass_kernel_spmd(trace=True) can capture NTFF profiles. Mirrors
    trn_agent_boot/trn_boot.py::_ntff_profile_via_ctypes."""
    import sys
    try:
        from antenv.axon_hooks import get_axon_ntff_profile_hook  # noqa: F401
        return
    except ImportError:
        pass
    import contextlib
    import ctypes
    import types

    so_path = "/opt/axon/libaxon_pjrt.so"
    hook = None
    try:
        lib = ctypes.CDLL(so_path)
        if hasattr(lib, "axon_start_nrt_profile"):
            lib.axon_start_nrt_profile.argtypes = [
                ctypes.POINTER(ctypes.c_int64), ctypes.c_size_t]
            lib.axon_start_nrt_profile.restype = ctypes.c_int64
            lib.axon_stop_nrt_profile.argtypes = [ctypes.c_char_p]
            lib.axon_stop_nrt_profile.restype = ctypes.c_int64

            @contextlib.contextmanager
            def _hook(output_dir, device_ids):
                import jax
                jax.devices()
                if device_ids:
                    ids = (ctypes.c_int64 * len(device_ids))(*device_ids)
                    rc = lib.axon_start_nrt_profile(ids, len(device_ids))
                else:
                    rc = lib.axon_start_nrt_profile(None, 0)
                if rc != 0:
                    raise RuntimeError(f"axon_start_nrt_profile rc={rc}")
                try:
                    yield
                finally:
                    n = lib.axon_stop_nrt_profile(str(output_dir).encode())
                    print(f"profile: {n} file(s) written to {output_dir}")

            hook = _hook
    except OSError:
        pass

    mod = types.ModuleType("antenv.axon_hooks")
    mod._hook = hook
    mod.get_axon_ntff_profile_hook = lambda: mod._hook
    mod.set_axon_ntff_profile_hook = lambda h: setattr(mod, "_hook", h)
    sys.modules["antenv.axon_hooks"] = mod


def run(inputs, trace=False):
    from concourse.bass_utils import run_bass_kernel_spmd
    if trace:
        _ensure_ntff_hook()
    nc = get_program()
    in_maps = make_in_maps(inputs)
    res = run_bass_kernel_spmd(nc, in_maps, core_ids=list(range(NCORES)),
                               trace=trace)
    acc = np.zeros((S, DIM), dtype=np.float32)
    for r in res.results:
        acc += np.asarray(r["out"], dtype=np.float32)
    return acc.reshape(B, S, DIM), res


def kernel(**inputs):
    out, _ = run(inputs, trace=False)
    return out
